# revision 30
# baseline (speedup 1.0000x reference)
"""EnsembleMLP fused kernel for Trainium2, 8 NeuronCores (SPMD, batch-parallel).

Math transformation
-------------------
reference:
    hidden = tanh(x @ W_in.T)                                   [B, H]
    feats[b,m,e] = hidden[b, ids[m,e]]                          [B, M, E]
    preds[b,m,o] = sum_e feats[b,m,e] * W_pred[m,o,e]           [B, M, O]
    out = preds.mean(axis=1)                                    [B, O]

The gather + per-member matmul + mean are all linear in `hidden`, so they
collapse into a single [H, O] matrix:
    A[h,o] = (1/M) * sum_{(m,e): ids[m,e]==h} W_pred[m,o,e]
    out    = tanh(x @ W_in.T) @ A

A is built on the host from the tiny W_pred/ids tensors (0.7 MB); the device
does the two matmuls + tanh. Sharding: data-parallel over batch — each of the
8 cores takes 512 rows of x; W_in^T and A are replicated. No collectives.

Device layout (per core)
------------------------
All DRAM inputs are host-packed partition-major ([128, free]) so every DMA
moves >=1KB-contiguous per-partition segments:
  xt  [128, 4*512]   bf16  x^T slice:  xt[p, n*512+b] = x[c*512+b, n*128+p]
  wt  [128, 32*512]  bf16  W_in^T:     wt[p, t*512+n*128+h] = W_in[t*128+h, n*128+p]
  aw  [128, 32*32]   bf16  A packed + zero-padded to 32 output cols:
                           aw[p, t*32+o] = A[t*128+p, o] for o<10 else 0
  out [10, 512]      f32   out^T slice (host transposes back)

Timeline design (from the v1 trace: 48.2us = 6.9us fixed NRT preamble +
5.1us DMA latency + 32.2us PE stream + 4.1us tail):
  - DMA issue order puts the 512KB x^T slice + wt tile 0 at the head of both
    HWDGE rings so the first real matmul can start ~10.4us (v1: 12.0us).
  - PE warm-up (HAM clock gate) starts the instant the Tensor preamble ends:
    first on a GpSimd-memset striped-constant tile, then on DVE random bits.
  - Ensemble matmuls are 4-way column-tiled (M=10 uses 10/128 PE columns;
    4 concurrent col groups at partition 0/32/64/96 of ONE psum bank) =>
    8 slots instead of 32.  A is zero-padded to 32 cols per h-tile so the
    whole psum bank is defined; the tail is ONE 128-partition ACT copy
    (psum->sbuf bf16) + one DMA, and the host sums the 4 col-group partials.
"""

import os

import numpy as np
import ml_dtypes

BATCH, IN_DIM, HIDDEN, N_MEMBERS, ENS, OUT = 4096, 512, 4096, 256, 64, 10
NCORES = 8
B_LOC = BATCH // NCORES      # 512 batch rows per core
HT = 128                     # h-tile height (PSUM partition dim)
NHT = HIDDEN // HT           # 32 h-tiles
NIC = IN_DIM // 128          # 4 contraction chunks for the first matmul
OPAD = 32                    # A columns padded 10 -> 32 (fills psum col group)
N_WARM_CONST = 8             # warm-up matmuls on the memset tile (earliest)
N_WARM = 28                  # warm-up matmuls on the random tile
# wt DMA group sizes (in h-tiles). Each HWDGE ring sustains only ~150-200
# GB/s early (v3 trace), and a group is usable only when its LAST byte +
# completion receipt land, so: small groups first (low latency), large ones
# later (amortize per-DMA cost), split across the two rings so each ring's
# cumulative bytes stay ahead of the 853ns/tile consumption.
WT_GROUPS = [2, 2, 4, 8, 8, 8]
assert sum(WT_GROUPS) == NHT
# Ensemble batch slices: big slice ships while the PE runs the small one.
H_SPLITS = [(0, 384), (384, B_LOC)]

_compiled = None
LAST_RESULT = None           # BassKernelResults of the most recent run


def _build_raw(num_devices=NCORES):
    """Hand-scheduled Bass version (no Tile framework).

    Engine programs:
      Sync   : ring A DMAs (xt, wt g1, g3); out DMA (batch slice 0)
      Scalar : ring B DMAs (wt g4, g5, aw); tanh PSUM->SBUF (bf16) per
               h-tile; final psum->sbuf copies; out DMA (batch slice 1)
      GpSimd : striped-constant memset of the early warm-up tile; SWDGE
               DMAs for wt g0, g2 (dodges ring B's table-load stall)
      PE     : warm-up matmuls (HAM clock-gate), 32x4 first-layer matmuls
               (psum bank t%4), then 32 ensemble matmuls 4-way column-tiled
               into one PSUM bank
      DVE    : random warm-up tile
    """
    from concourse import bacc, mybir

    bf16 = mybir.dt.bfloat16
    f32 = mybir.dt.float32

    nc = bacc.Bacc(
        "TRN2",
        target_bir_lowering=False,
        debug=False,
        enable_asserts=False,
        num_devices=num_devices,
    )
    xt = nc.dram_tensor("xt", [128, NIC * B_LOC], bf16, kind="ExternalInput")
    wt = nc.dram_tensor("wt", [128, NHT * NIC * HT], bf16, kind="ExternalInput")
    aw = nc.dram_tensor("aw", [128, NHT * OPAD], bf16, kind="ExternalInput")
    # out ships the 4 ensemble col-group partials (partitions 32j..32j+10);
    # the host does the final 4-way add + transpose.
    out = nc.dram_tensor("out", [128, B_LOC], bf16, kind="ExternalOutput")

    warm_c = nc.alloc_sbuf_tensor("warm_c", [128, 128], bf16)
    warm_sb = nc.alloc_sbuf_tensor("warm_sb", [128, 128], mybir.dt.uint16)
    dummy_sb = nc.alloc_sbuf_tensor("dummy_sb", [1, 16], f32)
    xt_sb = nc.alloc_sbuf_tensor("xt_sb", [128, NIC, B_LOC], bf16)
    wt_sb = [
        nc.alloc_sbuf_tensor(f"wt_g{g}", [128, k, NIC, HT], bf16)
        for g, k in enumerate(WT_GROUPS)
    ]
    ht_sb = [
        nc.alloc_sbuf_tensor(f"ht_sb{t}", [128, B_LOC], bf16) for t in range(NHT)
    ]
    a_sb = nc.alloc_sbuf_tensor("a_sb", [128, NHT * OPAD], bf16)
    out_sb = nc.alloc_sbuf_tensor("out_sb", [128, B_LOC], bf16)

    ps = [nc.alloc_psum_tensor(f"ps{k}", [128, B_LOC], f32) for k in range(4)]
    # One FULL bank per ensemble batch half: the half-0 ACT copy runs while
    # the PE accumulates half 1, and PE-write + ACT-read of the SAME psum
    # bank is a fatal HW collision — so the halves must be distinct banks.
    pso = [nc.alloc_psum_tensor(f"pso{h}", [128, B_LOC], f32) for h in range(2)]
    psw = nc.alloc_psum_tensor("psw", [128, B_LOC], f32)

    # Per-DMA completion semaphores: a DMA's +16 lands only on its own sem,
    # so waits are sound under any cross-DMA completion interleaving.
    s_xt = nc.alloc_semaphore("s_xt")
    s_wt = [nc.alloc_semaphore(f"s_wtg{g}") for g in range(len(WT_GROUPS))]
    s_aw = nc.alloc_semaphore("s_aw")
    s_out = nc.alloc_semaphore("s_out")
    sgc = nc.alloc_semaphore("sgc")  # constant warm tile memset done
    sg = nc.alloc_semaphore("sg")    # random warm tile ready (2 steps)
    sm = nc.alloc_semaphore("sm")    # first-layer tile t accumulated
    sa = nc.alloc_semaphore("sa")    # tanh t done
    sm2 = nc.alloc_semaphore("sm2")  # ensemble batch-half h done (inc per half)
    sv = nc.alloc_semaphore("sv")    # psum->sbuf copy of batch-half h done

    # tile index -> (group, index within group)
    tile_group = []
    for g, k in enumerate(WT_GROUPS):
        for i in range(k):
            tile_group.append((g, i))
    group_t0 = []
    t0 = 0
    for k in WT_GROUPS:
        group_t0.append(t0)
        t0 += k

    tanh = mybir.ActivationFunctionType.Tanh

    # ---- GpSimd: striped-constant warm tile, available the moment the
    # engine preambles end (~1.3us before the DVE random tile).  The PE HAM
    # clock-gate watches real datapath activity, so give the stripes some
    # variety; the DVE random tile takes over as soon as it is ready.
    for si, val in enumerate([1.0, 1.5, -1.25, 0.75]):
        mk = nc.gpsimd.memset(warm_c.ap()[:, si * 32 : (si + 1) * 32], val)
    mk.then_inc(sgc, 1)

    # ---- DVE: random bits masked to bf16 in [1, 2) ((bits&0x7F)|0x3F80).
    if os.environ.get("KERNEL_SIMSAFE") == "1":
        fill = nc.vector.memset(warm_sb.ap(), 0x3F80)  # CoreSim xorwow workaround
    else:
        fill = nc.vector.random(warm_sb.ap())
    fill.then_inc(sg, 1)
    nc.vector.wait_ge(sg, 1)              # DVE pipeline: fill retired
    nc.vector.tensor_scalar(
        out=warm_sb.ap(),
        in0=warm_sb.ap(),
        scalar1=0x007F,
        scalar2=0x3F80,
        op0=mybir.AluOpType.bitwise_and,
        op1=mybir.AluOpType.bitwise_or,
    ).then_inc(sg, 1)

    # ---- Input DMAs on three queues.  The Scalar HWDGE ring's kickoff is
    # blocked until the auto-hoisted ACT_TABLE_LOAD (tanh table, ~1.3us)
    # finishes, so its first bytes move ~1.5us after Sync's (v3/v4 traces):
    # early-critical data rides Sync (xt) and the independent GpSimd SWDGE
    # queue (wt t0-7); Scalar only carries late traffic.
    wt_view = wt.ap().rearrange("p (t n h) -> p t n h", t=NHT, n=NIC)

    def wt_dma(eng, g):
        k = WT_GROUPS[g]
        eng.dma_start(
            out=wt_sb[g].ap(), in_=wt_view[:, group_t0[g] : group_t0[g] + k, :, :]
        ).then_inc(s_wt[g], 16)

    # Sync (ring A): xt (512KB), then wt t2-3, t8-15
    nc.sync.dma_start(out=xt_sb.ap(), in_=xt.ap()).then_inc(s_xt, 16)
    wt_dma(nc.sync, 1)
    wt_dma(nc.sync, 3)
    # GpSimd (SWDGE): wt t0-1 (lands before xt), t4-7
    wt_dma(nc.gpsimd, 0)
    wt_dma(nc.gpsimd, 2)
    # Scalar (ring B): wt t16-23, t24-31, aw — none needed before ~25us
    wt_dma(nc.scalar, 4)
    wt_dma(nc.scalar, 5)
    nc.scalar.dma_start(out=a_sb.ap(), in_=aw.ap()).then_inc(s_aw, 16)

    # ---- PE
    pe = nc.tensor
    pe.wait_ge(sgc, 1)
    for _ in range(N_WARM_CONST):
        pe.matmul(
            out=psw.ap()[:, :128],
            lhsT=warm_c.ap(),
            rhs=warm_c.ap(),
            start=True,
            stop=True,
        )
    pe.wait_ge(sg, 2)
    warm_bf = warm_sb.ap().bitcast(bf16)
    for _ in range(N_WARM):
        pe.matmul(
            out=psw.ap()[:, :128],
            lhsT=warm_bf,
            rhs=warm_bf,
            start=True,
            stop=True,
        )
    pe.wait_ge(s_xt, 16)                      # xt landed
    for t in range(NHT):
        g, i = tile_group[t]
        if i == 0:
            pe.wait_ge(s_wt[g], 16)           # wt group g landed
        if t >= 4:
            pe.wait_ge(sa, t - 3)             # psum bank free after tanh(t-4)
        for n in range(NIC):
            mm = pe.matmul(
                out=ps[t % 4].ap(),
                lhsT=wt_sb[g].ap()[:, i, n, :],
                rhs=xt_sb.ap()[:, n, :],
                start=(n == 0),
                stop=(n == NIC - 1),
            )
        mm.then_inc(sm, 1)
    pe.wait_ge(s_aw, 16)                      # aw landed
    # Ensemble matmuls, 4-way column-tiled, split into two UNEVEN batch
    # slices: slice 0 (384 cols) copies + ships while the PE runs slice 1,
    # and the critical-path slice 1 (128 cols) keeps the final copy + DMA
    # small.  One full psum bank per slice (PE-write + ACT-read of the same
    # bank is a fatal HW collision).
    for h, (blo, bhi) in enumerate(H_SPLITS):
        for t in range(NHT):
            j = t % 4
            pe.wait_ge(sa, t + 1)             # ht tile t written
            mm = pe.matmul(
                out=pso[h].ap()[32 * j : 32 * j + OPAD, 0 : bhi - blo],
                lhsT=a_sb.ap()[:, t * OPAD : (t + 1) * OPAD],
                rhs=ht_sb[t].ap()[:, blo:bhi],
                start=(t < 4),
                stop=(t >= NHT - 4),
                tile_position=(0, 32 * j),
                # 4 interleaved accumulation groups share one psum bank at
                # disjoint partition ranges; has_written is per-element, so
                # this is HW-safe — the sim's zero-region check is coarser.
                skip_group_check=True,
            )
        # Col-tiled matmuls complete in pc order: one inc per slice is sound.
        mm.then_inc(sm2, 1)

    # ---- ACT: dummy first use pulls the act-table load off the critical path
    act = nc.scalar
    act.wait_ge(sgc, 1)
    act.activation(
        out=dummy_sb.ap(), in_=warm_c.ap()[:1, :16], func=tanh
    )
    for t in range(NHT):
        act.wait_ge(sm, t + 1)
        act.activation(out=ht_sb[t].ap(), in_=ps[t % 4].ap(), func=tanh).then_inc(
            sa, 1
        )
    # Per batch slice: one 128-partition PSUM->SBUF copy of the 4 col-group
    # partials (bf16); the host sums the groups. (DVE/ACT can read at most
    # one PSUM input per instruction, so an on-device reduce would cost a
    # 3-op chain.)  Slice 0 copies + ships while the PE runs slice 1.
    for h, (blo, bhi) in enumerate(H_SPLITS):
        act.wait_ge(sm2, h + 1)
        act.activation(
            out=out_sb.ap()[:, blo:bhi],
            in_=pso[h].ap()[:, 0 : bhi - blo],
            func=mybir.ActivationFunctionType.Copy,
        ).then_inc(sv, 1)

    # ---- Tail: result out per batch slice, on SEPARATE HWDGE rings so the
    # two completions don't serialize (per-ring FIFO).  No explicit
    # completion wait or sem reset: the NRT-injected per-engine epilogue
    # drains every queue and resets the whole semaphore file.
    blo, bhi = H_SPLITS[0]
    nc.sync.wait_ge(sv, 1)
    nc.sync.dma_start(
        out=out.ap()[:, blo:bhi], in_=out_sb.ap()[:, blo:bhi]
    ).then_inc(s_out, 16)
    blo, bhi = H_SPLITS[1]
    nc.scalar.wait_ge(sv, 2)
    nc.scalar.dma_start(
        out=out.ap()[:, blo:bhi], in_=out_sb.ap()[:, blo:bhi]
    ).then_inc(s_out, 16)

    nc.compile()
    return nc


def _host_pack(x, W_in, W_pred, ids):
    # Collapse gather + einsum + mean into A[h, o], zero-padded to OPAD cols.
    A = np.zeros((HIDDEN, OUT), dtype=np.float64)
    np.add.at(
        A,
        ids.reshape(-1),
        W_pred.transpose(0, 2, 1).reshape(-1, OUT).astype(np.float64),
    )
    A /= N_MEMBERS
    Ap = np.zeros((NHT, 128, OPAD), dtype=np.float64)
    Ap[:, :, :OUT] = A.reshape(NHT, 128, OUT)
    a_packed = np.ascontiguousarray(
        Ap.transpose(1, 0, 2).reshape(128, NHT * OPAD)
    ).astype(ml_dtypes.bfloat16)

    xt_bf = x.T.astype(ml_dtypes.bfloat16)                     # [512, 4096]
    wt_bf = W_in.T.astype(ml_dtypes.bfloat16)                  # [512, 4096]
    # wt packed partition-major: [p, t*512 + n*128 + h] = W_in.T[n*128+p, t*128+h]
    wt_packed = np.ascontiguousarray(
        wt_bf.reshape(NIC, 128, NHT, HT).transpose(1, 2, 0, 3).reshape(128, -1)
    )
    return xt_bf, wt_packed, a_packed


def kernel(**inputs) -> np.ndarray:
    x = np.asarray(inputs["x"], dtype=np.float32)              # [4096, 512]
    W_in = np.asarray(inputs["W_in"], dtype=np.float32)        # [4096, 512]
    W_pred = np.asarray(inputs["W_pred"], dtype=np.float32)    # [256, 10, 64]
    ids = np.asarray(inputs["ensemble_input_ids"])             # [256, 64] int32

    xt_bf, wt_packed, a_packed = _host_pack(x, W_in, W_pred, ids)

    global _compiled
    if _compiled is None:
        _compiled = _build_raw()
    nc = _compiled

    in_maps = []
    for c in range(NCORES):
        xs = xt_bf[:, c * B_LOC : (c + 1) * B_LOC]             # [512, 512]
        xt_packed = np.ascontiguousarray(
            xs.reshape(NIC, 128, B_LOC).transpose(1, 0, 2).reshape(128, -1)
        )
        in_maps.append({"xt": xt_packed, "wt": wt_packed, "aw": a_packed})

    from concourse.bass_utils import run_bass_kernel_spmd

    trace = bool(int(os.environ.get("KERNEL_TRACE", "0")))
    res = run_bass_kernel_spmd(
        nc, in_maps, core_ids=list(range(NCORES)), trace=trace
    )
    global LAST_RESULT
    LAST_RESULT = res

    out = np.empty((BATCH, OUT), dtype=np.float32)
    for c in range(NCORES):
        p = res.results[c]["out"].astype(np.float32)           # [128, 512]
        s = p[0:OUT] + p[32 : 32 + OUT] + p[64 : 64 + OUT] + p[96 : 96 + OUT]
        out[c * B_LOC : (c + 1) * B_LOC, :] = s.T
    return out


# revision 33
# speedup vs baseline: 1.0968x; 1.0968x over previous
"""EnsembleMLP fused kernel for Trainium2, 8 NeuronCores (SPMD, batch-parallel).

Math transformation
-------------------
reference:
    hidden = tanh(x @ W_in.T)                                   [B, H]
    feats[b,m,e] = hidden[b, ids[m,e]]                          [B, M, E]
    preds[b,m,o] = sum_e feats[b,m,e] * W_pred[m,o,e]           [B, M, O]
    out = preds.mean(axis=1)                                    [B, O]

The gather + per-member matmul + mean are all linear in `hidden`, so they
collapse into a single [H, O] matrix:
    A[h,o] = (1/M) * sum_{(m,e): ids[m,e]==h} W_pred[m,o,e]
    out    = tanh(x @ W_in.T) @ A

A is built on the host from the tiny W_pred/ids tensors (0.7 MB); the device
does the two matmuls + tanh. Sharding: data-parallel over batch — each of the
8 cores takes 512 rows of x; W_in^T and A are replicated. No collectives.

Device layout (per core)
------------------------
All DRAM inputs are host-packed partition-major ([128, free]) so every DMA
moves >=1KB-contiguous per-partition segments:
  xt  [128, 4*512]   bf16  x^T slice:  xt[p, n*512+b] = x[c*512+b, n*128+p]
  wt  [128, 32*512]  bf16  W_in^T:     wt[p, t*512+n*128+h] = W_in[t*128+h, n*128+p]
  aw  [128, 32*32]   bf16  A packed + zero-padded to 32 output cols:
                           aw[p, t*32+o] = A[t*128+p, o] for o<10 else 0
  out [10, 512]      f32   out^T slice (host transposes back)

Timeline design (from the v1 trace: 48.2us = 6.9us fixed NRT preamble +
5.1us DMA latency + 32.2us PE stream + 4.1us tail):
  - DMA issue order puts the 512KB x^T slice + wt tile 0 at the head of both
    HWDGE rings so the first real matmul can start ~10.4us (v1: 12.0us).
  - PE warm-up (HAM clock gate) starts the instant the Tensor preamble ends:
    first on a GpSimd-memset striped-constant tile, then on DVE random bits.
  - Ensemble matmuls are 4-way column-tiled (M=10 uses 10/128 PE columns;
    4 concurrent col groups at partition 0/32/64/96 of ONE psum bank) =>
    8 slots instead of 32.  A is zero-padded to 32 cols per h-tile so the
    whole psum bank is defined; the tail is ONE 128-partition ACT copy
    (psum->sbuf bf16) + one DMA, and the host sums the 4 col-group partials.
"""

import os

import numpy as np
import ml_dtypes

BATCH, IN_DIM, HIDDEN, N_MEMBERS, ENS, OUT = 4096, 512, 4096, 256, 64, 10
NCORES = 8
B_LOC = BATCH // NCORES      # 512 batch rows per core
HT = 128                     # h-tile height (PSUM partition dim)
NHT = HIDDEN // HT           # 32 h-tiles
NIC = IN_DIM // 128          # 4 contraction chunks for the first matmul
OPAD = 32                    # A columns padded 10 -> 32 (fills psum col group)
N_WARM_CONST = 8             # warm-up matmuls on the memset tile (earliest)
N_WARM = 31                  # warm-up matmuls on the random tile
# wt DMA group sizes (in h-tiles). The input path is AGGREGATE-bandwidth
# capped (~300 GB/s = HBM-per-NC limit) across all DMA queues, so early
# bandwidth must never go to late-needed bytes: everything rides ONE ring
# (Sync) in strict need order — xt, then wt groups sized so each lands
# just ahead of the 853ns/tile consumption.  Scalar's ring only carries
# aw (needed at ~38us) because its kickoff trails Sync's by ~1.5us (the
# auto-hoisted tanh ACT_TABLE_LOAD blocks it).
WT_GROUPS = [2, 3, 4, 6, 8, 9]
assert sum(WT_GROUPS) == NHT
# Ensemble batch slices: big slice ships while the PE runs the small one.
H_SPLITS = [(0, 384), (384, B_LOC)]

_compiled = None
LAST_RESULT = None           # BassKernelResults of the most recent run


def _build_raw(num_devices=NCORES):
    """Hand-scheduled Bass version (no Tile framework).

    Engine programs:
      Sync   : ALL input DMAs (xt + wt groups, strict need order);
               out DMA (batch slice 0)
      Scalar : aw DMA (late-needed); tanh PSUM->SBUF (bf16) per h-tile;
               final psum->sbuf copies; out DMA (batch slice 1)
      GpSimd : striped-constant memset of the early warm-up tile
      PE     : warm-up matmuls (HAM clock-gate), 32x4 first-layer matmuls
               (psum bank t%4), then 32 ensemble matmuls 4-way column-tiled
               into one PSUM bank
      DVE    : random warm-up tile
    """
    from concourse import bacc, mybir

    bf16 = mybir.dt.bfloat16
    f32 = mybir.dt.float32

    nc = bacc.Bacc(
        "TRN2",
        target_bir_lowering=False,
        debug=False,
        enable_asserts=False,
        num_devices=num_devices,
    )
    xt = nc.dram_tensor("xt", [128, NIC * B_LOC], bf16, kind="ExternalInput")
    wt = nc.dram_tensor("wt", [128, NHT * NIC * HT], bf16, kind="ExternalInput")
    aw = nc.dram_tensor("aw", [128, NHT * OPAD], bf16, kind="ExternalInput")
    # out ships the 4 ensemble col-group partials (partitions 32j..32j+10);
    # the host does the final 4-way add + transpose.
    out = nc.dram_tensor("out", [128, B_LOC], bf16, kind="ExternalOutput")

    warm_c = nc.alloc_sbuf_tensor("warm_c", [128, 128], bf16)
    warm_sb = nc.alloc_sbuf_tensor("warm_sb", [128, 128], mybir.dt.uint16)
    dummy_sb = nc.alloc_sbuf_tensor("dummy_sb", [1, 16], f32)
    xt_sb = nc.alloc_sbuf_tensor("xt_sb", [128, NIC, B_LOC], bf16)
    wt_sb = [
        nc.alloc_sbuf_tensor(f"wt_g{g}", [128, k, NIC, HT], bf16)
        for g, k in enumerate(WT_GROUPS)
    ]
    ht_sb = [
        nc.alloc_sbuf_tensor(f"ht_sb{t}", [128, B_LOC], bf16) for t in range(NHT)
    ]
    a_sb = nc.alloc_sbuf_tensor("a_sb", [128, NHT * OPAD], bf16)
    out_sb = nc.alloc_sbuf_tensor("out_sb", [128, B_LOC], bf16)

    ps = [nc.alloc_psum_tensor(f"ps{k}", [128, B_LOC], f32) for k in range(4)]
    # One FULL bank per ensemble batch half: the half-0 ACT copy runs while
    # the PE accumulates half 1, and PE-write + ACT-read of the SAME psum
    # bank is a fatal HW collision — so the halves must be distinct banks.
    pso = [nc.alloc_psum_tensor(f"pso{h}", [128, B_LOC], f32) for h in range(2)]
    psw = nc.alloc_psum_tensor("psw", [128, B_LOC], f32)

    # Per-DMA completion semaphores: a DMA's +16 lands only on its own sem,
    # so waits are sound under any cross-DMA completion interleaving.
    s_xt = nc.alloc_semaphore("s_xt")
    s_wt = [nc.alloc_semaphore(f"s_wtg{g}") for g in range(len(WT_GROUPS))]
    s_aw = nc.alloc_semaphore("s_aw")
    s_out = nc.alloc_semaphore("s_out")
    sgc = nc.alloc_semaphore("sgc")  # constant warm tile memset done
    sg = nc.alloc_semaphore("sg")    # random warm tile ready (2 steps)
    sm = nc.alloc_semaphore("sm")    # first-layer tile t accumulated
    sa = nc.alloc_semaphore("sa")    # tanh t done
    sm2 = nc.alloc_semaphore("sm2")  # ensemble batch-half h done (inc per half)
    sv = nc.alloc_semaphore("sv")    # psum->sbuf copy of batch-half h done

    # tile index -> (group, index within group)
    tile_group = []
    for g, k in enumerate(WT_GROUPS):
        for i in range(k):
            tile_group.append((g, i))
    group_t0 = []
    t0 = 0
    for k in WT_GROUPS:
        group_t0.append(t0)
        t0 += k

    tanh = mybir.ActivationFunctionType.Tanh

    # ---- GpSimd: striped-constant warm tile, available the moment the
    # engine preambles end (~1.3us before the DVE random tile).  The PE HAM
    # clock-gate watches real datapath activity, so give the stripes some
    # variety; the DVE random tile takes over as soon as it is ready.
    for si, val in enumerate([1.0, 1.5, -1.25, 0.75]):
        mk = nc.gpsimd.memset(warm_c.ap()[:, si * 32 : (si + 1) * 32], val)
    mk.then_inc(sgc, 1)

    # ---- DVE: random bits masked to bf16 in [1, 2) ((bits&0x7F)|0x3F80).
    if os.environ.get("KERNEL_SIMSAFE") == "1":
        fill = nc.vector.memset(warm_sb.ap(), 0x3F80)  # CoreSim xorwow workaround
    else:
        fill = nc.vector.random(warm_sb.ap())
    fill.then_inc(sg, 1)
    nc.vector.wait_ge(sg, 1)              # DVE pipeline: fill retired
    nc.vector.tensor_scalar(
        out=warm_sb.ap(),
        in0=warm_sb.ap(),
        scalar1=0x007F,
        scalar2=0x3F80,
        op0=mybir.AluOpType.bitwise_and,
        op1=mybir.AluOpType.bitwise_or,
    ).then_inc(sg, 1)

    # ---- Input DMAs: ONE ring (Sync) in strict need order; Scalar carries
    # only the late-needed aw.  Any bytes moving early on another queue
    # would steal aggregate HBM bandwidth from the critical head (v5 trace).
    wt_view = wt.ap().rearrange("p (t n h) -> p t n h", t=NHT, n=NIC)

    def wt_dma(eng, g):
        k = WT_GROUPS[g]
        eng.dma_start(
            out=wt_sb[g].ap(), in_=wt_view[:, group_t0[g] : group_t0[g] + k, :, :]
        ).then_inc(s_wt[g], 16)

    nc.sync.dma_start(out=xt_sb.ap(), in_=xt.ap()).then_inc(s_xt, 16)
    for g in range(len(WT_GROUPS)):
        wt_dma(nc.sync, g)
    nc.scalar.dma_start(out=a_sb.ap(), in_=aw.ap()).then_inc(s_aw, 16)

    # ---- PE
    pe = nc.tensor
    pe.wait_ge(sgc, 1)
    for _ in range(N_WARM_CONST):
        pe.matmul(
            out=psw.ap()[:, :128],
            lhsT=warm_c.ap(),
            rhs=warm_c.ap(),
            start=True,
            stop=True,
        )
    pe.wait_ge(sg, 2)
    warm_bf = warm_sb.ap().bitcast(bf16)
    for _ in range(N_WARM):
        pe.matmul(
            out=psw.ap()[:, :128],
            lhsT=warm_bf,
            rhs=warm_bf,
            start=True,
            stop=True,
        )
    pe.wait_ge(s_xt, 16)                      # xt landed
    for t in range(NHT):
        g, i = tile_group[t]
        if i == 0:
            pe.wait_ge(s_wt[g], 16)           # wt group g landed
        if t >= 4:
            pe.wait_ge(sa, t - 3)             # psum bank free after tanh(t-4)
        for n in range(NIC):
            mm = pe.matmul(
                out=ps[t % 4].ap(),
                lhsT=wt_sb[g].ap()[:, i, n, :],
                rhs=xt_sb.ap()[:, n, :],
                start=(n == 0),
                stop=(n == NIC - 1),
            )
        mm.then_inc(sm, 1)
    pe.wait_ge(s_aw, 16)                      # aw landed
    # Ensemble matmuls, 4-way column-tiled, split into two UNEVEN batch
    # slices: slice 0 (384 cols) copies + ships while the PE runs slice 1,
    # and the critical-path slice 1 (128 cols) keeps the final copy + DMA
    # small.  One full psum bank per slice (PE-write + ACT-read of the same
    # bank is a fatal HW collision).
    for h, (blo, bhi) in enumerate(H_SPLITS):
        for t in range(NHT):
            j = t % 4
            pe.wait_ge(sa, t + 1)             # ht tile t written
            mm = pe.matmul(
                out=pso[h].ap()[32 * j : 32 * j + OPAD, 0 : bhi - blo],
                lhsT=a_sb.ap()[:, t * OPAD : (t + 1) * OPAD],
                rhs=ht_sb[t].ap()[:, blo:bhi],
                start=(t < 4),
                stop=(t >= NHT - 4),
                tile_position=(0, 32 * j),
                # 4 interleaved accumulation groups share one psum bank at
                # disjoint partition ranges; has_written is per-element, so
                # this is HW-safe — the sim's zero-region check is coarser.
                skip_group_check=True,
            )
        # Col-tiled matmuls complete in pc order: one inc per slice is sound.
        mm.then_inc(sm2, 1)

    # ---- ACT: dummy first use pulls the act-table load off the critical path
    act = nc.scalar
    act.wait_ge(sgc, 1)
    act.activation(
        out=dummy_sb.ap(), in_=warm_c.ap()[:1, :16], func=tanh
    )
    for t in range(NHT):
        act.wait_ge(sm, t + 1)
        act.activation(out=ht_sb[t].ap(), in_=ps[t % 4].ap(), func=tanh).then_inc(
            sa, 1
        )
    # Per batch slice: one 128-partition PSUM->SBUF copy of the 4 col-group
    # partials (bf16); the host sums the groups. (DVE/ACT can read at most
    # one PSUM input per instruction, so an on-device reduce would cost a
    # 3-op chain.)  Slice 0 copies + ships while the PE runs slice 1.
    for h, (blo, bhi) in enumerate(H_SPLITS):
        act.wait_ge(sm2, h + 1)
        act.activation(
            out=out_sb.ap()[:, blo:bhi],
            in_=pso[h].ap()[:, 0 : bhi - blo],
            func=mybir.ActivationFunctionType.Copy,
        ).then_inc(sv, 1)

    # ---- Tail: result out per batch slice, on SEPARATE HWDGE rings so the
    # two completions don't serialize (per-ring FIFO).  No explicit
    # completion wait or sem reset: the NRT-injected per-engine epilogue
    # drains every queue and resets the whole semaphore file.
    blo, bhi = H_SPLITS[0]
    nc.sync.wait_ge(sv, 1)
    nc.sync.dma_start(
        out=out.ap()[:, blo:bhi], in_=out_sb.ap()[:, blo:bhi]
    ).then_inc(s_out, 16)
    blo, bhi = H_SPLITS[1]
    nc.scalar.wait_ge(sv, 2)
    nc.scalar.dma_start(
        out=out.ap()[:, blo:bhi], in_=out_sb.ap()[:, blo:bhi]
    ).then_inc(s_out, 16)

    nc.compile()
    return nc


def _host_pack(x, W_in, W_pred, ids):
    # Collapse gather + einsum + mean into A[h, o], zero-padded to OPAD cols.
    A = np.zeros((HIDDEN, OUT), dtype=np.float64)
    np.add.at(
        A,
        ids.reshape(-1),
        W_pred.transpose(0, 2, 1).reshape(-1, OUT).astype(np.float64),
    )
    A /= N_MEMBERS
    Ap = np.zeros((NHT, 128, OPAD), dtype=np.float64)
    Ap[:, :, :OUT] = A.reshape(NHT, 128, OUT)
    a_packed = np.ascontiguousarray(
        Ap.transpose(1, 0, 2).reshape(128, NHT * OPAD)
    ).astype(ml_dtypes.bfloat16)

    xt_bf = x.T.astype(ml_dtypes.bfloat16)                     # [512, 4096]
    wt_bf = W_in.T.astype(ml_dtypes.bfloat16)                  # [512, 4096]
    # wt packed partition-major: [p, t*512 + n*128 + h] = W_in.T[n*128+p, t*128+h]
    wt_packed = np.ascontiguousarray(
        wt_bf.reshape(NIC, 128, NHT, HT).transpose(1, 2, 0, 3).reshape(128, -1)
    )
    return xt_bf, wt_packed, a_packed


def kernel(**inputs) -> np.ndarray:
    x = np.asarray(inputs["x"], dtype=np.float32)              # [4096, 512]
    W_in = np.asarray(inputs["W_in"], dtype=np.float32)        # [4096, 512]
    W_pred = np.asarray(inputs["W_pred"], dtype=np.float32)    # [256, 10, 64]
    ids = np.asarray(inputs["ensemble_input_ids"])             # [256, 64] int32

    xt_bf, wt_packed, a_packed = _host_pack(x, W_in, W_pred, ids)

    global _compiled
    if _compiled is None:
        _compiled = _build_raw()
    nc = _compiled

    in_maps = []
    for c in range(NCORES):
        xs = xt_bf[:, c * B_LOC : (c + 1) * B_LOC]             # [512, 512]
        xt_packed = np.ascontiguousarray(
            xs.reshape(NIC, 128, B_LOC).transpose(1, 0, 2).reshape(128, -1)
        )
        in_maps.append({"xt": xt_packed, "wt": wt_packed, "aw": a_packed})

    from concourse.bass_utils import run_bass_kernel_spmd

    trace = bool(int(os.environ.get("KERNEL_TRACE", "0")))
    res = run_bass_kernel_spmd(
        nc, in_maps, core_ids=list(range(NCORES)), trace=trace
    )
    global LAST_RESULT
    LAST_RESULT = res

    out = np.empty((BATCH, OUT), dtype=np.float32)
    for c in range(NCORES):
        p = res.results[c]["out"].astype(np.float32)           # [128, 512]
        s = p[0:OUT] + p[32 : 32 + OUT] + p[64 : 64 + OUT] + p[96 : 96 + OUT]
        out[c * B_LOC : (c + 1) * B_LOC, :] = s.T
    return out


# revision 37
# speedup vs baseline: 1.1186x; 1.0199x over previous
"""EnsembleMLP fused kernel for Trainium2, 8 NeuronCores (SPMD, batch-parallel).

Math transformation
-------------------
reference:
    hidden = tanh(x @ W_in.T)                                   [B, H]
    feats[b,m,e] = hidden[b, ids[m,e]]                          [B, M, E]
    preds[b,m,o] = sum_e feats[b,m,e] * W_pred[m,o,e]           [B, M, O]
    out = preds.mean(axis=1)                                    [B, O]

The gather + per-member matmul + mean are all linear in `hidden`, so they
collapse into a single [H, O] matrix:
    A[h,o] = (1/M) * sum_{(m,e): ids[m,e]==h} W_pred[m,o,e]
    out    = tanh(x @ W_in.T) @ A

A is built on the host from the tiny W_pred/ids tensors (0.7 MB); the device
does the two matmuls + tanh. Sharding: data-parallel over batch — each of the
8 cores takes 512 rows of x; W_in^T and A are replicated. No collectives.

Device layout (per core)
------------------------
All DRAM inputs are host-packed partition-major ([128, free]) so every DMA
moves >=1KB-contiguous per-partition segments:
  xt  [128, 4*512]   bf16  x^T slice:  xt[p, n*512+b] = x[c*512+b, n*128+p]
  wt  [128, 32*512]  bf16  W_in^T:     wt[p, t*512+n*128+h] = W_in[t*128+h, n*128+p]
  aw  [128, 32*32]   bf16  A packed + zero-padded to 32 output cols:
                           aw[p, t*32+o] = A[t*128+p, o] for o<10 else 0
  out [10, 512]      f32   out^T slice (host transposes back)

Timeline design (from the v1 trace: 48.2us = 6.9us fixed NRT preamble +
5.1us DMA latency + 32.2us PE stream + 4.1us tail):
  - DMA issue order puts the 512KB x^T slice + wt tile 0 at the head of both
    HWDGE rings so the first real matmul can start ~10.4us (v1: 12.0us).
  - PE warm-up (HAM clock gate) starts the instant the Tensor preamble ends:
    first on a GpSimd-memset striped-constant tile, then on DVE random bits.
  - Ensemble matmuls are 4-way column-tiled (M=10 uses 10/128 PE columns;
    4 concurrent col groups at partition 0/32/64/96 of ONE psum bank) =>
    8 slots instead of 32.  A is zero-padded to 32 cols per h-tile so the
    whole psum bank is defined; the tail is ONE 128-partition ACT copy
    (psum->sbuf bf16) + one DMA, and the host sums the 4 col-group partials.
"""

import os

import numpy as np
import ml_dtypes

BATCH, IN_DIM, HIDDEN, N_MEMBERS, ENS, OUT = 4096, 512, 4096, 256, 64, 10
NCORES = 8
B_LOC = BATCH // NCORES      # 512 batch rows per core
HT = 128                     # h-tile height (PSUM partition dim)
NHT = HIDDEN // HT           # 32 h-tiles
NIC = IN_DIM // 128          # 4 contraction chunks for the first matmul
OPAD = 32                    # A columns padded 10 -> 32 (fills psum col group)
N_WARM_CONST = 8             # warm-up matmuls on the memset tile (earliest)
N_WARM = 34                  # warm-up matmuls on the random tile
# wt DMA group sizes (in h-tiles). Measured DMA behavior (v2-v6 traces):
# each HWDGE ring caps at ~0.17 MB/us (Sync's first byte ~8.4us; Scalar's
# ~10us — its kickoff waits for the auto-hoisted tanh ACT_TABLE_LOAD), the
# two rings together reach the ~300 GB/s HBM limit, and a transfer is
# usable only at last-byte + receipt.  So: strict need order split across
# both rings, small groups first, and NO late-needed bytes early (aw is
# issued mid-tanh from Scalar so it cannot steal head bandwidth).
WT_GROUPS = [2, 2, 4, 7, 8, 9]
assert sum(WT_GROUPS) == NHT
# Ensemble batch slices: big slice ships while the PE runs the small one.
H_SPLITS = [(0, 384), (384, B_LOC)]

_compiled = None
LAST_RESULT = None           # BassKernelResults of the most recent run


def _build_raw(num_devices=NCORES):
    """Hand-scheduled Bass version (no Tile framework).

    Engine programs:
      Sync   : ring A input DMAs (xt, wt g1, g3, g5); out DMA (slice 0)
      Scalar : ring B input DMAs (wt g0, g2, g4; aw mid-tanh); tanh
               PSUM->SBUF (bf16) per h-tile; final psum->sbuf copies;
               out DMA (batch slice 1)
      GpSimd : striped-constant memset of the early warm-up tile
      PE     : warm-up matmuls (HAM clock-gate), 32x4 first-layer matmuls
               (psum bank t%4), then 32 ensemble matmuls 4-way column-tiled
               into one PSUM bank
      DVE    : random warm-up tile
    """
    from concourse import bacc, mybir

    bf16 = mybir.dt.bfloat16
    f32 = mybir.dt.float32

    nc = bacc.Bacc(
        "TRN2",
        target_bir_lowering=False,
        debug=False,
        enable_asserts=False,
        num_devices=num_devices,
    )
    xt = nc.dram_tensor("xt", [128, NIC * B_LOC], bf16, kind="ExternalInput")
    wt = nc.dram_tensor("wt", [128, NHT * NIC * HT], bf16, kind="ExternalInput")
    aw = nc.dram_tensor("aw", [128, NHT * OPAD], bf16, kind="ExternalInput")
    # out ships the 4 ensemble col-group partials (partitions 32j..32j+10);
    # the host does the final 4-way add + transpose.
    out = nc.dram_tensor("out", [128, B_LOC], bf16, kind="ExternalOutput")

    warm_c = nc.alloc_sbuf_tensor("warm_c", [128, 128], bf16)
    warm_sb = nc.alloc_sbuf_tensor("warm_sb", [128, 128], mybir.dt.uint16)
    dummy_sb = nc.alloc_sbuf_tensor("dummy_sb", [1, 16], f32)
    xt_sb = nc.alloc_sbuf_tensor("xt_sb", [128, NIC, B_LOC], bf16)
    wt_sb = [
        nc.alloc_sbuf_tensor(f"wt_g{g}", [128, k, NIC, HT], bf16)
        for g, k in enumerate(WT_GROUPS)
    ]
    ht_sb = [
        nc.alloc_sbuf_tensor(f"ht_sb{t}", [128, B_LOC], bf16) for t in range(NHT)
    ]
    a_sb = nc.alloc_sbuf_tensor("a_sb", [128, NHT * OPAD], bf16)
    out_sb = nc.alloc_sbuf_tensor("out_sb", [128, B_LOC], bf16)

    ps = [nc.alloc_psum_tensor(f"ps{k}", [128, B_LOC], f32) for k in range(4)]
    # One FULL bank per ensemble batch half: the half-0 ACT copy runs while
    # the PE accumulates half 1, and PE-write + ACT-read of the SAME psum
    # bank is a fatal HW collision — so the halves must be distinct banks.
    pso = [nc.alloc_psum_tensor(f"pso{h}", [128, B_LOC], f32) for h in range(2)]
    psw = nc.alloc_psum_tensor("psw", [128, B_LOC], f32)

    # Per-DMA completion semaphores: a DMA's +16 lands only on its own sem,
    # so waits are sound under any cross-DMA completion interleaving.
    s_xt = nc.alloc_semaphore("s_xt")
    s_wt = [nc.alloc_semaphore(f"s_wtg{g}") for g in range(len(WT_GROUPS))]
    s_aw = nc.alloc_semaphore("s_aw")
    s_out = nc.alloc_semaphore("s_out")
    sgc = nc.alloc_semaphore("sgc")  # constant warm tile memset done
    sg = nc.alloc_semaphore("sg")    # random warm tile ready (2 steps)
    sm = nc.alloc_semaphore("sm")    # first-layer tile t accumulated
    sa = nc.alloc_semaphore("sa")    # tanh t done
    sm2 = nc.alloc_semaphore("sm2")  # ensemble batch-half h done (inc per half)
    sv = nc.alloc_semaphore("sv")    # psum->sbuf copy of batch-half h done

    # tile index -> (group, index within group)
    tile_group = []
    for g, k in enumerate(WT_GROUPS):
        for i in range(k):
            tile_group.append((g, i))
    group_t0 = []
    t0 = 0
    for k in WT_GROUPS:
        group_t0.append(t0)
        t0 += k

    tanh = mybir.ActivationFunctionType.Tanh

    # ---- GpSimd: striped-constant warm tile, available the moment the
    # engine preambles end (~1.3us before the DVE random tile).  The PE HAM
    # clock-gate watches real datapath activity, so give the stripes some
    # variety; the DVE random tile takes over as soon as it is ready.
    for si, val in enumerate([1.0, 1.5, -1.25, 0.75]):
        mk = nc.gpsimd.memset(warm_c.ap()[:, si * 32 : (si + 1) * 32], val)
    mk.then_inc(sgc, 1)

    # ---- DVE: random bits masked to bf16 in [1, 2) ((bits&0x7F)|0x3F80).
    if os.environ.get("KERNEL_SIMSAFE") == "1":
        fill = nc.vector.memset(warm_sb.ap(), 0x3F80)  # CoreSim xorwow workaround
    else:
        fill = nc.vector.random(warm_sb.ap())
    fill.then_inc(sg, 1)
    nc.vector.wait_ge(sg, 1)              # DVE pipeline: fill retired
    nc.vector.tensor_scalar(
        out=warm_sb.ap(),
        in0=warm_sb.ap(),
        scalar1=0x007F,
        scalar2=0x3F80,
        op0=mybir.AluOpType.bitwise_and,
        op1=mybir.AluOpType.bitwise_or,
    ).then_inc(sg, 1)

    # ---- Input DMAs in strict need order across both HWDGE rings.
    wt_view = wt.ap().rearrange("p (t n h) -> p t n h", t=NHT, n=NIC)

    def wt_dma(eng, g):
        k = WT_GROUPS[g]
        eng.dma_start(
            out=wt_sb[g].ap(), in_=wt_view[:, group_t0[g] : group_t0[g] + k, :, :]
        ).then_inc(s_wt[g], 16)

    # Ring A (Sync): xt, wt t2-3, t8-14, t23-31
    nc.sync.dma_start(out=xt_sb.ap(), in_=xt.ap()).then_inc(s_xt, 16)
    wt_dma(nc.sync, 1)
    wt_dma(nc.sync, 3)
    wt_dma(nc.sync, 5)
    # Ring B (Scalar): wt t0-1, t4-7, t15-22.  aw is issued later, from
    # inside the tanh sequence (see the ACT section).
    wt_dma(nc.scalar, 0)
    wt_dma(nc.scalar, 2)
    wt_dma(nc.scalar, 4)

    # ---- PE
    pe = nc.tensor
    pe.wait_ge(sgc, 1)
    for _ in range(N_WARM_CONST):
        pe.matmul(
            out=psw.ap()[:, :128],
            lhsT=warm_c.ap(),
            rhs=warm_c.ap(),
            start=True,
            stop=True,
        )
    pe.wait_ge(sg, 2)
    warm_bf = warm_sb.ap().bitcast(bf16)
    for _ in range(N_WARM):
        pe.matmul(
            out=psw.ap()[:, :128],
            lhsT=warm_bf,
            rhs=warm_bf,
            start=True,
            stop=True,
        )
    pe.wait_ge(s_xt, 16)                      # xt landed
    for t in range(NHT):
        g, i = tile_group[t]
        if i == 0:
            pe.wait_ge(s_wt[g], 16)           # wt group g landed
        if t >= 4:
            pe.wait_ge(sa, t - 3)             # psum bank free after tanh(t-4)
        for n in range(NIC):
            mm = pe.matmul(
                out=ps[t % 4].ap(),
                lhsT=wt_sb[g].ap()[:, i, n, :],
                rhs=xt_sb.ap()[:, n, :],
                start=(n == 0),
                stop=(n == NIC - 1),
            )
        mm.then_inc(sm, 1)
    pe.wait_ge(s_aw, 16)                      # aw landed
    # Ensemble matmuls, 4-way column-tiled, split into two UNEVEN batch
    # slices: slice 0 (384 cols) copies + ships while the PE runs slice 1,
    # and the critical-path slice 1 (128 cols) keeps the final copy + DMA
    # small.  One full psum bank per slice (PE-write + ACT-read of the same
    # bank is a fatal HW collision).
    for h, (blo, bhi) in enumerate(H_SPLITS):
        for t in range(NHT):
            j = t % 4
            pe.wait_ge(sa, t + 1)             # ht tile t written
            mm = pe.matmul(
                out=pso[h].ap()[32 * j : 32 * j + OPAD, 0 : bhi - blo],
                lhsT=a_sb.ap()[:, t * OPAD : (t + 1) * OPAD],
                rhs=ht_sb[t].ap()[:, blo:bhi],
                start=(t < 4),
                stop=(t >= NHT - 4),
                tile_position=(0, 32 * j),
                # 4 interleaved accumulation groups share one psum bank at
                # disjoint partition ranges; has_written is per-element, so
                # this is HW-safe — the sim's zero-region check is coarser.
                skip_group_check=True,
            )
        # Col-tiled matmuls complete in pc order: one inc per slice is sound.
        mm.then_inc(sm2, 1)

    # ---- ACT: dummy first use pulls the act-table load off the critical path
    act = nc.scalar
    act.wait_ge(sgc, 1)
    act.activation(
        out=dummy_sb.ap(), in_=warm_c.ap()[:1, :16], func=tanh
    )
    for t in range(NHT):
        act.wait_ge(sm, t + 1)
        act.activation(out=ht_sb[t].ap(), in_=ps[t % 4].ap(), func=tanh).then_inc(
            sa, 1
        )
        if t == 4:
            # aw (needed only by the ~38us ensemble phase) is issued here so
            # its bytes cannot steal early HBM bandwidth from the xt/wt head.
            nc.scalar.dma_start(out=a_sb.ap(), in_=aw.ap()).then_inc(s_aw, 16)
    # Per batch slice: one 128-partition PSUM->SBUF copy of the 4 col-group
    # partials (bf16); the host sums the groups. (DVE/ACT can read at most
    # one PSUM input per instruction, so an on-device reduce would cost a
    # 3-op chain.)  Slice 0 copies + ships while the PE runs slice 1.
    for h, (blo, bhi) in enumerate(H_SPLITS):
        act.wait_ge(sm2, h + 1)
        act.activation(
            out=out_sb.ap()[:, blo:bhi],
            in_=pso[h].ap()[:, 0 : bhi - blo],
            func=mybir.ActivationFunctionType.Copy,
        ).then_inc(sv, 1)

    # ---- Tail: result out per batch slice, on SEPARATE HWDGE rings so the
    # two completions don't serialize (per-ring FIFO).  No explicit
    # completion wait or sem reset: the NRT-injected per-engine epilogue
    # drains every queue and resets the whole semaphore file.
    blo, bhi = H_SPLITS[0]
    nc.sync.wait_ge(sv, 1)
    nc.sync.dma_start(
        out=out.ap()[:, blo:bhi], in_=out_sb.ap()[:, blo:bhi]
    ).then_inc(s_out, 16)
    blo, bhi = H_SPLITS[1]
    nc.scalar.wait_ge(sv, 2)
    nc.scalar.dma_start(
        out=out.ap()[:, blo:bhi], in_=out_sb.ap()[:, blo:bhi]
    ).then_inc(s_out, 16)

    nc.compile()
    return nc


def _host_pack(x, W_in, W_pred, ids):
    # Collapse gather + einsum + mean into A[h, o], zero-padded to OPAD cols.
    A = np.zeros((HIDDEN, OUT), dtype=np.float64)
    np.add.at(
        A,
        ids.reshape(-1),
        W_pred.transpose(0, 2, 1).reshape(-1, OUT).astype(np.float64),
    )
    A /= N_MEMBERS
    Ap = np.zeros((NHT, 128, OPAD), dtype=np.float64)
    Ap[:, :, :OUT] = A.reshape(NHT, 128, OUT)
    a_packed = np.ascontiguousarray(
        Ap.transpose(1, 0, 2).reshape(128, NHT * OPAD)
    ).astype(ml_dtypes.bfloat16)

    xt_bf = x.T.astype(ml_dtypes.bfloat16)                     # [512, 4096]
    wt_bf = W_in.T.astype(ml_dtypes.bfloat16)                  # [512, 4096]
    # wt packed partition-major: [p, t*512 + n*128 + h] = W_in.T[n*128+p, t*128+h]
    wt_packed = np.ascontiguousarray(
        wt_bf.reshape(NIC, 128, NHT, HT).transpose(1, 2, 0, 3).reshape(128, -1)
    )
    return xt_bf, wt_packed, a_packed


def kernel(**inputs) -> np.ndarray:
    x = np.asarray(inputs["x"], dtype=np.float32)              # [4096, 512]
    W_in = np.asarray(inputs["W_in"], dtype=np.float32)        # [4096, 512]
    W_pred = np.asarray(inputs["W_pred"], dtype=np.float32)    # [256, 10, 64]
    ids = np.asarray(inputs["ensemble_input_ids"])             # [256, 64] int32

    xt_bf, wt_packed, a_packed = _host_pack(x, W_in, W_pred, ids)

    global _compiled
    if _compiled is None:
        _compiled = _build_raw()
    nc = _compiled

    in_maps = []
    for c in range(NCORES):
        xs = xt_bf[:, c * B_LOC : (c + 1) * B_LOC]             # [512, 512]
        xt_packed = np.ascontiguousarray(
            xs.reshape(NIC, 128, B_LOC).transpose(1, 0, 2).reshape(128, -1)
        )
        in_maps.append({"xt": xt_packed, "wt": wt_packed, "aw": a_packed})

    from concourse.bass_utils import run_bass_kernel_spmd

    trace = bool(int(os.environ.get("KERNEL_TRACE", "0")))
    res = run_bass_kernel_spmd(
        nc, in_maps, core_ids=list(range(NCORES)), trace=trace
    )
    global LAST_RESULT
    LAST_RESULT = res

    out = np.empty((BATCH, OUT), dtype=np.float32)
    for c in range(NCORES):
        p = res.results[c]["out"].astype(np.float32)           # [128, 512]
        s = p[0:OUT] + p[32 : 32 + OUT] + p[64 : 64 + OUT] + p[96 : 96 + OUT]
        out[c * B_LOC : (c + 1) * B_LOC, :] = s.T
    return out
